# revision 1
# baseline (speedup 1.0000x reference)
"""GPT decoder layer (B=2,T=2048,D=1024,H=16,KS=64,FF=4096, partial rotary 32,
RMSNorm, causal, SwiGLU) on 8 trn2 NeuronCores.

Sharding: core c handles batch b=c//4, query block qs=(c%4)*512 (512 tokens).
No collectives: each core computes K/V for its full batch (replicated weights),
attention + FFN only for its 512-token block. Fully "transposed" dataflow:
feature dims on partitions, tokens on free axis -> no on-chip transposes.
"""
import numpy as np
import ml_dtypes

import concourse.bass as bass
import concourse.tile as tile
from concourse import bacc, mybir
from concourse._compat import with_exitstack

F32 = mybir.dt.float32
BF16 = mybir.dt.bfloat16
AF = mybir.ActivationFunctionType

B, T, D, H, KS, FF, ROT = 2, 2048, 1024, 16, 64, 4096, 32
P = 128
NCORES = 8
QB = 512          # query-block tokens per core
NDT = D // P      # 8 dim tiles
NTT = T // P      # 16 token tiles
EPS = 1e-6

BF = ml_dtypes.bfloat16


# ---------------------------------------------------------------- host prep
def _rope_tables():
    """cos/sin tables in transposed layout (128 partitions = 2 heads x 64 dims,
    tokens on free axis), with the even/odd columns PERMUTED to rotate-half
    form (we permute Wq/Wk columns to match)."""
    inv_freq = 1.0 / (10000 ** (np.arange(0, ROT, 2) / ROT))       # (16,)
    t = np.arange(T)
    ang = np.einsum("i,j->ij", t, inv_freq)                         # (T,16)
    sin, cos = np.sin(ang), np.cos(ang)                             # (T,16)
    # rows 0..15: j=0..15 (original even dims), rows 16..31: same (odd dims)
    # rows 32..63: pass-through, cos=1 sin=0
    cos64 = np.ones((64, T), np.float32)
    sin64 = np.zeros((64, T), np.float32)
    cos64[0:16] = cos.T
    cos64[16:32] = cos.T
    sin64[0:16] = sin.T
    sin64[16:32] = sin.T
    cosT = np.concatenate([cos64, cos64], axis=0)                   # (128,T)
    sinT = np.concatenate([sin64, sin64], axis=0)
    return cosT, sinT


def _pmatT():
    """lhsT for rot-half: out = Pmat @ x, out[j]=-x[16+j], out[16+j]=x[j]
    (j=0..15 within each 64-dim head block), rows 32..63 -> 0."""
    Pm = np.zeros((128, 128), np.float32)
    for base in (0, 64):
        for j in range(16):
            Pm[base + j, base + 16 + j] = -1.0
            Pm[base + 16 + j, base + j] = 1.0
    return np.ascontiguousarray(Pm.T)  # lhsT = Pmat.T


def _perm_cols(w):
    """Permute first ROT dims of each head's columns to even-first order."""
    w = w.copy()
    perm = np.concatenate([np.arange(0, ROT, 2), np.arange(1, ROT, 2)])
    for h in range(H):
        base = h * KS
        w[:, base:base + ROT] = w[:, base + perm]
    return w


def prep_inputs(x, attention_mask, Wq, Wk, Wv, Wo, attn_scale, ffn_scale, W1, W2):
    """Build the 8 per-core input maps (shared arrays are reused by reference)."""
    asc = attn_scale.astype(np.float32)[:, None]
    fsc = ffn_scale.astype(np.float32)[:, None]
    wq = _perm_cols(asc * Wq.astype(np.float32) / np.sqrt(KS)).astype(BF)
    wk = _perm_cols(asc * Wk.astype(np.float32)).astype(BF)
    wv = (asc * Wv.astype(np.float32)).astype(BF)
    wo = Wo.astype(np.float32).astype(BF)
    w1 = (fsc * W1.astype(np.float32)).astype(BF)
    w2 = W2.astype(np.float32).astype(BF)
    cosT, sinT = _rope_tables()
    coskT = cosT.astype(BF)
    sinkT = sinT.astype(BF)
    pmT = _pmatT().astype(BF)
    m = np.asarray(attention_mask[0, 0])                            # (T,T) bool
    xTb = [np.ascontiguousarray(np.asarray(x[b]).T.astype(np.float32))
           for b in range(B)]                                       # (D,T)
    in_maps = []
    for c in range(NCORES):
        b, qs = c // 4, (c % 4) * QB
        # rotate tokens so this core's query block sits at columns 0..QB-1;
        # attention is permutation-invariant over keys (mask + K-rope tables
        # are permuted consistently).
        perm = np.concatenate([np.arange(qs, T), np.arange(0, qs)])
        in_maps.append({
            "xT": np.ascontiguousarray(xTb[b][:, perm]),
            "mask01T": np.ascontiguousarray(m[qs:qs + QB, :][:, perm].T
                                            ).astype(BF),
            "wq": wq, "wk": wk, "wv": wv, "wo": wo, "w1": w1, "w2": w2,
            "coskT": np.ascontiguousarray(coskT[:, perm]),
            "sinkT": np.ascontiguousarray(sinkT[:, perm]),
            "cosqT": np.ascontiguousarray(coskT[:, qs:qs + QB]),
            "sinqT": np.ascontiguousarray(sinkT[:, qs:qs + QB]),
            "pmT": pmT,
        })
    return in_maps


# ---------------------------------------------------------------- device code

# ---------------------------------------------------------------- device code
@with_exitstack
def decoder_kernel(ctx, tc):
    nc = tc.nc
    xT = nc.dram_tensor("xT", [D, T], F32, kind="ExternalInput").ap()
    mask01T = nc.dram_tensor("mask01T", [T, QB], BF16, kind="ExternalInput").ap()
    wq_d = nc.dram_tensor("wq", [D, D], BF16, kind="ExternalInput").ap()
    wk_d = nc.dram_tensor("wk", [D, D], BF16, kind="ExternalInput").ap()
    wv_d = nc.dram_tensor("wv", [D, D], BF16, kind="ExternalInput").ap()
    wo_d = nc.dram_tensor("wo", [D, D], BF16, kind="ExternalInput").ap()
    w1_d = nc.dram_tensor("w1", [D, 2 * FF], BF16, kind="ExternalInput").ap()
    w2_d = nc.dram_tensor("w2", [FF, D], BF16, kind="ExternalInput").ap()
    coskT_d = nc.dram_tensor("coskT", [P, T], BF16, kind="ExternalInput").ap()
    sinkT_d = nc.dram_tensor("sinkT", [P, T], BF16, kind="ExternalInput").ap()
    cosqT_d = nc.dram_tensor("cosqT", [P, QB], BF16, kind="ExternalInput").ap()
    sinqT_d = nc.dram_tensor("sinqT", [P, QB], BF16, kind="ExternalInput").ap()
    pmT_d = nc.dram_tensor("pmT", [P, P], BF16, kind="ExternalInput").ap()
    outT = nc.dram_tensor("outT", [D, QB], F32, kind="ExternalOutput").ap()

    # ---- long-lived pool: rotary const + attn out + h/hn (KB/part: ~34)
    pers = ctx.enter_context(tc.tile_pool(name="pers", bufs=1))
    pmT = pers.tile([P, P], BF16, name="pmT", tag="pmT")
    nc.sync.dma_start(pmT[:], pmT_d[:])
    aT = [pers.tile([P, QB], BF16, name=f"aT{k}", tag=f"aT{k}")
          for k in range(NDT)]
    hT = [pers.tile([P, QB], F32, name=f"hT{k}", tag=f"hT{k}")
          for k in range(NDT)]

    def rmsnorm_rstd(pool, src_tiles, ntok, tag):
        """returns rstd tile (P, ntok) f32 (same value on all partitions)."""
        acc = pool.tile([P, ntok], F32, name=f"{tag}acc", tag=f"{tag}acc")
        tmp = pool.tile([P, ntok], F32, name=f"{tag}tmp", tag=f"{tag}tmp")
        nc.vector.tensor_mul(acc[:], src_tiles[0][:], src_tiles[0][:])
        for k in range(1, NDT):
            nc.vector.tensor_mul(tmp[:], src_tiles[k][:], src_tiles[k][:])
            nc.vector.tensor_add(acc[:], acc[:], tmp[:])
        red = pool.tile([P, ntok], F32, name=f"{tag}red", tag=f"{tag}red")
        nc.gpsimd.partition_all_reduce(red[:], acc[:], channels=P,
                                       reduce_op=bass.bass_isa.ReduceOp.add)
        # rstd = sqrt(1/(ssq/D + eps))
        nc.vector.tensor_scalar(red[:], red[:], 1.0 / D, EPS,
                                op0=mybir.AluOpType.mult,
                                op1=mybir.AluOpType.add)
        nc.vector.reciprocal(red[:], red[:])
        nc.scalar.activation(red[:], red[:], AF.Sqrt)
        return red

    def rotary(psum_pool, tmp_pool, kps, raw_tag, out_tile, col0, n,
               cos_t, sin_t, idx):
        """kps: psum (P,n) K/Q proj result; writes rotated bf16 into
        out_tile[:, col0:col0+n]."""
        raw = tmp_pool.tile([P, n], BF16, name=f"{raw_tag}raw",
                            tag=f"{raw_tag}raw")
        nc.scalar.copy(raw[:], kps[:])
        rps = psum_pool.tile([P, n], F32, name=f"{raw_tag}rps",
                             tag=f"{raw_tag}rps")
        nc.tensor.matmul(rps[:], pmT[:], raw[:], start=True, stop=True)
        t1 = tmp_pool.tile([P, n], F32, name=f"{raw_tag}t1", tag=f"{raw_tag}t1")
        nc.vector.tensor_mul(t1[:], raw[:], cos_t[:, idx * n:(idx + 1) * n])
        t2 = tmp_pool.tile([P, n], F32, name=f"{raw_tag}t2", tag=f"{raw_tag}t2")
        nc.vector.tensor_mul(t2[:], rps[:], sin_t[:, idx * n:(idx + 1) * n])
        nc.vector.tensor_add(out_tile[:, col0:col0 + n], t1[:], t2[:])

    NTB = T // QB   # 4 token blocks of 512
    NFT = FF // P   # 32 f-tiles per FFN half

    # ================= scope B: attention working set (~82 KB/part)
    with tc.tile_pool(name="attnB", bufs=1) as attnB:
        kT = [attnB.tile([P, T], BF16, name=f"kT{k}", tag=f"kT{k}")
              for k in range(NDT)]
        qT = [attnB.tile([P, QB], BF16, name=f"qT{k}", tag=f"qT{k}")
              for k in range(NDT)]
        vaug = [attnB.tile([P, H * (KS + 1)], BF16, name=f"va{t}",
                           tag=f"va{t}") for t in range(NTT)]
        cosk = attnB.tile([P, T], BF16, name="cosk", tag="cosk")
        sink = attnB.tile([P, T], BF16, name="sink", tag="sink")
        cosq = attnB.tile([P, QB], BF16, name="cosq", tag="cosq")
        sinq = attnB.tile([P, QB], BF16, name="sinq", tag="sinq")
        nc.sync.dma_start(cosk[:], coskT_d[:])
        nc.sync.dma_start(sink[:], sinkT_d[:])
        nc.sync.dma_start(cosq[:], cosqT_d[:])
        nc.sync.dma_start(sinq[:], sinqT_d[:])

        # ============= scope A: xnT bf16 (full batch) (~32 KB/part)
        with tc.tile_pool(name="attnA", bufs=1) as attnA:
            xnT = [attnA.tile([P, T], BF16, name=f"xnT{k}", tag=f"xnT{k}")
                   for k in range(NDT)]
            xnB = [t[:, 0:QB] for t in xnT]  # query block = first QB tokens

            # ---- phase 0: stream x, accumulate sum-of-squares, cast to bf16
            # into xnT, then scale in place by rstd.
            with tc.tile_pool(name="ph0", bufs=1) as ph0:
                acc = ph0.tile([P, T], F32, name="nacc", tag="nacc")
                tmp = ph0.tile([P, T], F32, name="ntmp", tag="ntmp")
                for k in range(NDT):
                    xs = ph0.tile([P, T], F32, name=f"xs{k % 2}",
                                  tag=f"xs{k % 2}")
                    nc.sync.dma_start(xs[:], xT[k * P:(k + 1) * P, :])
                    if k == 0:
                        nc.vector.tensor_mul(acc[:], xs[:], xs[:])
                    else:
                        nc.vector.tensor_mul(tmp[:], xs[:], xs[:])
                        nc.vector.tensor_add(acc[:], acc[:], tmp[:])
                    nc.vector.tensor_copy(xnT[k][:], xs[:])
                red = ph0.tile([P, T], F32, name="nred", tag="nred")
                nc.gpsimd.partition_all_reduce(red[:], acc[:], channels=P,
                                               reduce_op=bass.bass_isa.ReduceOp.add)
                nc.vector.tensor_scalar(red[:], red[:], 1.0 / D, EPS,
                                        op0=mybir.AluOpType.mult,
                                        op1=mybir.AluOpType.add)
                nc.vector.reciprocal(red[:], red[:])
                nc.scalar.activation(red[:], red[:], AF.Sqrt)
                for k in range(NDT):
                    nc.vector.tensor_mul(xnT[k][:], xnT[k][:], red[:])

            # ---- phase 1: projections (weight tags shared across matrices)
            with tc.tile_pool(name="wsc", bufs=1) as wsc, \
                 tc.tile_pool(name="rotw", bufs=2) as rotw:

                def load_w(dram):
                    ws = []
                    for k in range(NDT):
                        w = wsc.tile([P, D], BF16, name=f"w{k}", tag=f"w{k}")
                        nc.sync.dma_start(w[:], dram[k * P:(k + 1) * P, :])
                        ws.append(w)
                    return ws

                # Q^T
                with tc.tile_pool(name="p1q", bufs=2, space="PSUM") as p1q:
                    wq = load_w(wq_d)
                    for dt in range(NDT):
                        qps = p1q.tile([P, QB], F32, name="qps", tag="qps")
                        for k in range(NDT):
                            nc.tensor.matmul(qps[:],
                                             wq[k][:, dt * P:(dt + 1) * P],
                                             xnB[k][:], start=(k == 0),
                                             stop=(k == NDT - 1))
                        rotary(p1q, rotw, qps, "q", qT[dt], 0, QB,
                               cosq, sinq, 0)

                # K^T
                with tc.tile_pool(name="p1k", bufs=1, space="PSUM") as p1k:
                    wk = load_w(wk_d)
                    for dt in range(NDT):
                        kps = [p1k.tile([P, QB], F32, name=f"kps{tb}",
                                        tag=f"kps{tb}") for tb in range(NTB)]
                        for k in range(NDT):
                            for tb in range(NTB):
                                nc.tensor.matmul(
                                    kps[tb][:], wk[k][:, dt * P:(dt + 1) * P],
                                    xnT[k][:, tb * QB:(tb + 1) * QB],
                                    start=(k == 0), stop=(k == NDT - 1))
                        for tb in range(NTB):
                            rotary(p1k, rotw, kps[tb], "k", kT[dt], tb * QB,
                                   QB, cosk, sink, tb)

                # V natural + ones column per head
                with tc.tile_pool(name="p1v", bufs=2, space="PSUM") as p1v:
                    wv = load_w(wv_d)
                    for tt in range(NTT):
                        vps = [p1v.tile([P, QB], F32, name=f"vps{j}",
                                        tag=f"vps{j}") for j in range(2)]
                        for k in range(NDT):
                            for j in range(2):
                                nc.tensor.matmul(
                                    vps[j][:],
                                    xnT[k][:, tt * P:(tt + 1) * P],
                                    wv[k][:, j * QB:(j + 1) * QB],
                                    start=(k == 0), stop=(k == NDT - 1))
                        va3 = vaug[tt].rearrange("p (h e) -> p h e", e=KS + 1)
                        nc.vector.memset(va3[:, :, KS:KS + 1], 1.0)
                        for j in range(2):
                            v3 = vps[j].rearrange("p (h e) -> p h e", e=KS)
                            nc.vector.tensor_copy(va3[:, j * 8:(j + 1) * 8,
                                                      0:KS], v3[:])
        # scope A closed (xnT freed)

        # ---- phase 2: attention -> aT
        with tc.tile_pool(name="msc", bufs=1) as msc, \
             tc.tile_pool(name="p2", bufs=1, space="PSUM") as p2, \
             tc.tile_pool(name="p2s", bufs=3) as p2s:
            mask = []
            for t in range(NTT):
                m_ = msc.tile([P, QB], BF16, name=f"m{t}", tag=f"m{t}")
                nc.sync.dma_start(m_[:], mask01T[t * P:(t + 1) * P, :])
                mask.append(m_)
            for h in range(H):
                dt, row = h // 2, (h % 2) * KS
                avps = p2.tile([KS + 1, QB], F32, name=f"avps{h % 2}",
                               tag=f"avps{h % 2}")
                for kt in range(NTT):
                    sps = p2.tile([P, QB], F32, name=f"sps{kt % 3}",
                                  tag=f"sps{kt % 3}")
                    nc.tensor.matmul(sps[:],
                                     kT[dt][row:row + KS, kt * P:(kt + 1) * P],
                                     qT[dt][row:row + KS, :],
                                     start=True, stop=True)
                    e = p2s.tile([P, QB], BF16, name="e", tag="e")
                    nc.scalar.activation(e[:], sps[:], AF.Exp)
                    em = p2s.tile([P, QB], BF16, name="em", tag="em")
                    nc.vector.tensor_mul(em[:], e[:], mask[kt][:])
                    va3 = vaug[kt].rearrange("p (h e) -> p h e", e=KS + 1)
                    nc.tensor.matmul(avps[:], va3[:, h, :], em[:],
                                     start=(kt == 0), stop=(kt == NTT - 1),
                                     skip_group_check=True)
                rec = p2s.tile([1, QB], F32, name="rec", tag="rec")
                nc.vector.reciprocal(rec[:], avps[KS:KS + 1, :])
                recb = p2s.tile([KS, QB], F32, name="recb", tag="recb")
                nc.gpsimd.partition_broadcast(recb[:], rec[:])
                nc.vector.tensor_mul(aT[dt][row:row + KS, :], avps[0:KS, :],
                                     recb[:])

        # ---- phase 3: O-proj + residual (inside scope B to keep LIFO)
        with tc.tile_pool(name="p3w", bufs=1) as p3w, \
             tc.tile_pool(name="p3", bufs=1, space="PSUM") as p3:
            wo = []
            for k in range(NDT):
                w = p3w.tile([P, D], BF16, name=f"wo{k}", tag=f"wo{k}")
                nc.sync.dma_start(w[:], wo_d[k * P:(k + 1) * P, :])
                wo.append(w)
            xb2 = []
            for k in range(NDT):
                t_ = p3w.tile([P, QB], F32, name=f"xb2{k}", tag=f"xb2{k}")
                nc.sync.dma_start(t_[:], xT[k * P:(k + 1) * P, 0:QB])
                xb2.append(t_)
            for dt in range(NDT):
                ops = p3.tile([P, QB], F32, name=f"ops{dt % 2}",
                              tag=f"ops{dt % 2}")
                for k in range(NDT):
                    nc.tensor.matmul(ops[:], wo[k][:, dt * P:(dt + 1) * P],
                                     aT[k][:], start=(k == 0),
                                     stop=(k == NDT - 1))
                nc.vector.tensor_add(hT[dt][:], ops[:], xb2[dt][:])
    # scope B closed

    # ---- phase 3b: rmsnorm(h) -> hnT
    with tc.tile_pool(name="ffn", bufs=1) as ffn:
        hnT = [ffn.tile([P, QB], BF16, name=f"hnT{k}", tag=f"hnT{k}")
               for k in range(NDT)]
        with tc.tile_pool(name="p3n", bufs=1) as p3n:
            redh = rmsnorm_rstd(p3n, hT, QB, "hn")
            for k in range(NDT):
                nc.vector.tensor_mul(hnT[k][:], hT[k][:], redh[:])

        # ---- phase 4: FFN (SwiGLU) + residual -> outT
        gT = [ffn.tile([P, QB], BF16, name=f"gT{f}", tag=f"gT{f}")
              for f in range(NFT)]
        FG = 8  # f-tiles per DMA chunk group
        with tc.tile_pool(name="p4h", bufs=2, space="PSUM") as p4h, \
             tc.tile_pool(name="w1p", bufs=1) as w1p, \
             tc.tile_pool(name="p4s", bufs=3) as p4s:
            for fg in range(NFT // FG):
                w1a = [w1p.tile([P, FG * P], BF16, name=f"w1a{k}",
                                tag=f"w1a{k}") for k in range(NDT)]
                w1b = [w1p.tile([P, FG * P], BF16, name=f"w1b{k}",
                                tag=f"w1b{k}") for k in range(NDT)]
                c0 = fg * FG * P
                for k in range(NDT):
                    nc.sync.dma_start(w1a[k][:],
                                      w1_d[k * P:(k + 1) * P, c0:c0 + FG * P])
                    nc.sync.dma_start(
                        w1b[k][:],
                        w1_d[k * P:(k + 1) * P, FF + c0:FF + c0 + FG * P])
                for fi in range(FG):
                    f = fg * FG + fi
                    h1 = p4h.tile([P, QB], F32, name="h1", tag="h1")
                    h2 = p4h.tile([P, QB], F32, name="h2", tag="h2")
                    for k in range(NDT):
                        nc.tensor.matmul(h1[:], w1a[k][:, fi * P:(fi + 1) * P],
                                         hnT[k][:], start=(k == 0),
                                         stop=(k == NDT - 1))
                    for k in range(NDT):
                        nc.tensor.matmul(h2[:], w1b[k][:, fi * P:(fi + 1) * P],
                                         hnT[k][:], start=(k == 0),
                                         stop=(k == NDT - 1))
                    s1 = p4s.tile([P, QB], BF16, name="s1", tag="s1")
                    nc.scalar.activation(s1[:], h1[:], AF.Sigmoid)
                    t1 = p4s.tile([P, QB], F32, name="t1", tag="t1")
                    nc.vector.tensor_mul(t1[:], h1[:], s1[:])
                    nc.vector.tensor_mul(gT[f][:], t1[:], h2[:])

        with tc.tile_pool(name="p4o", bufs=1, space="PSUM") as p4o, \
             tc.tile_pool(name="w2p", bufs=1) as w2p, \
             tc.tile_pool(name="p4os", bufs=2) as p4os:
            w2 = []
            for f in range(NFT):
                w = w2p.tile([P, D], BF16, name=f"w2{f}", tag=f"w2{f}")
                nc.sync.dma_start(w[:], w2_d[f * P:(f + 1) * P, :])
                w2.append(w)
            for dt in range(NDT):
                fps = p4o.tile([P, QB], F32, name=f"fps{dt % 2}",
                               tag=f"fps{dt % 2}")
                for f in range(NFT):
                    nc.tensor.matmul(fps[:], w2[f][:, dt * P:(dt + 1) * P],
                                     gT[f][:], start=(f == 0),
                                     stop=(f == NFT - 1))
                o = p4os.tile([P, QB], F32, name="o", tag="o")
                nc.vector.tensor_add(o[:], fps[:], hT[dt][:])
                nc.sync.dma_start(outT[dt * P:(dt + 1) * P, :], o[:])


# ---------------------------------------------------------------- driver
_CACHE = {}


def build_nc():
    if "nc" in _CACHE:
        return _CACHE["nc"]
    nc = bacc.Bacc("TRN2", target_bir_lowering=False, debug=False,
                   enable_asserts=False)
    with tile.TileContext(nc) as tc:
        decoder_kernel(tc)
    nc.compile()
    _CACHE["nc"] = nc
    return nc


def kernel(x, attention_mask, Wq, Wk, Wv, Wo, attn_scale, ffn_scale, W1, W2,
           trace=False):
    from concourse import bass_utils
    in_maps = prep_inputs(x, attention_mask, Wq, Wk, Wv, Wo,
                          attn_scale, ffn_scale, W1, W2)
    nc = build_nc()
    res = bass_utils.run_bass_kernel_spmd(nc, in_maps,
                                          core_ids=list(range(NCORES)),
                                          trace=trace)
    out = np.empty((B, T, D), np.float32)
    for c in range(NCORES):
        b, qs = c // 4, (c % 4) * QB
        out[b, qs:qs + QB, :] = res.results[c]["outT"].T
    _CACHE["last_result"] = res
    return out



# revision 15
# speedup vs baseline: 1.8368x; 1.8368x over previous
"""GPT decoder layer (B=2,T=2048,D=1024,H=16,KS=64,FF=4096, partial rotary 32,
RMSNorm, causal, SwiGLU) on 8 trn2 NeuronCores.

Sharding: core c handles batch b=c//4, queries {s+4j : j=0..511} with s=c%4
(token-striped -> uniform causal structure across cores, ~47% of the dense
score/exp work). No collectives. fp8e4m3 DoubleRow matmuls (2x contraction,
0.5 cyc/col) for all projections + FFN + AV; weights pre-scaled x64 with 1/64
folded into per-token multiplies. RMSNorm folded into rope tables (rstd*cos,
rstd*sin) and V via per-partition scalars, so raw x feeds the projections
directly.
"""
import numpy as np
import ml_dtypes

import concourse.bass as bass
import concourse.tile as tile
from concourse import bacc, mybir
from concourse._compat import with_exitstack

F32 = mybir.dt.float32
BF16 = mybir.dt.bfloat16
FP8 = mybir.dt.float8e4
U8 = mybir.dt.uint8
AF = mybir.ActivationFunctionType
DR = mybir.MatmulPerfMode.DoubleRow

B, T, D, H, KS, FF, ROT = 2, 2048, 1024, 16, 64, 4096, 32
P = 128
NCORES = 8
QB = 512            # queries per core
NDT = D // P        # 8 dim tiles
NKP = D // 256      # 4 contraction pairs over D
EPS = 1e-6
SW = 64.0           # weight scale for fp8
GS = 8.0            # extra scale on GLU product

Bb = ml_dtypes.bfloat16
F8 = ml_dtypes.float8_e4m3


# ---------------------------------------------------------------- host prep
def _rope_tables():
    """cos/sin tables (128 partitions = 2 heads x 64 dims, tokens free),
    even/odd columns permuted to rotate-half form (wq/wk cols permuted to
    match)."""
    inv_freq = 1.0 / (10000 ** (np.arange(0, ROT, 2) / ROT))        # (16,)
    t = np.arange(T)
    ang = np.einsum("i,j->ij", t, inv_freq)                          # (T,16)
    sin, cos = np.sin(ang), np.cos(ang)
    cos64 = np.ones((64, T), np.float32)
    sin64 = np.zeros((64, T), np.float32)
    cos64[0:16] = cos.T
    cos64[16:32] = cos.T
    sin64[0:16] = sin.T
    sin64[16:32] = sin.T
    cosT = np.concatenate([cos64, cos64], axis=0)                    # (128,T)
    sinT = np.concatenate([sin64, sin64], axis=0)
    return cosT, sinT


def _pmat():
    """Pm: rot-half on the permuted layout. out = Pm @ x per 128-dim block:
    out[j] = -x[16+j], out[16+j] = x[j] (j=0..15 per 64-block), rows 32..63
    of each block -> 0."""
    Pm = np.zeros((128, 128), np.float32)
    for base in (0, 64):
        for j in range(16):
            Pm[base + j, base + 16 + j] = -1.0
            Pm[base + 16 + j, base + j] = 1.0
    return Pm


def _perm_cols(w):
    """Permute first ROT dims of each head's columns to even-first order."""
    w = w.copy()
    perm = np.concatenate([np.arange(0, ROT, 2), np.arange(1, ROT, 2)])
    for h in range(H):
        base = h * KS
        w[:, base:base + ROT] = w[:, base + perm]
    return w


def _pair8(w):
    """(K, M) f32 -> fp8 pair layout [128, K//256, M//128, 2, 128]:
    w8[p, kp, mt, j, m] = w[256kp+128j+p, 128mt+m] (contiguous [2,128]
    blocks as dual-fp8 ldweights requires)."""
    K, M = w.shape
    return np.ascontiguousarray(
        w.reshape(K // 256, 2, 128, M // 128, 128).transpose(2, 0, 3, 1, 4)
    ).astype(F8)


def prep_inputs(x, attention_mask, Wq, Wk, Wv, Wo, attn_scale, ffn_scale, W1, W2):
    asc = attn_scale.astype(np.float32)[:, None]
    fsc = ffn_scale.astype(np.float32)[:, None]
    Pm = _pmat()
    wq = _perm_cols(asc * Wq.astype(np.float32) / np.sqrt(KS)) * SW
    wk = _perm_cols(asc * Wk.astype(np.float32)) * SW
    # rot-folded weights: rot_out = Pm @ (W^T x) = (W @ Pm^T)^T x, per block
    def rotfold(w):
        wr = np.empty_like(w)
        for dtb in range(NDT):
            blk = slice(dtb * P, (dtb + 1) * P)
            wr[:, blk] = w[:, blk] @ Pm.T
        return wr
    wq8m, wq8r = _pair8(wq), _pair8(rotfold(wq))
    wk8m, wk8r = _pair8(wk), _pair8(rotfold(wk))
    wv8 = _pair8(asc * Wv.astype(np.float32) * SW)
    wo8 = _pair8(Wo.astype(np.float32) * SW)
    # W1 with x1/x2 column interleave: for f in 0..31: [x1 f-block | x2 f-block]
    w1s = (fsc * W1.astype(np.float32) * SW)
    w1i = np.empty_like(w1s)
    for f in range(FF // P):
        w1i[:, 256 * f:256 * f + 128] = w1s[:, 128 * f:128 * f + 128]
        w1i[:, 256 * f + 128:256 * f + 256] = w1s[:, FF + 128 * f:FF + 128 * f + 128]
    w18 = _pair8(w1i)
    w28 = _pair8(W2.astype(np.float32) * SW)
    cosT, sinT = _rope_tables()
    coskT = cosT.astype(Bb)
    sinkT = sinT.astype(Bb)
    ident = np.eye(P, dtype=np.float32).astype(Bb)
    xTb = [np.ascontiguousarray(np.asarray(x[b]).T.astype(np.float32))
           for b in range(B)]                                        # (D,T)
    in_maps = []
    for c in range(NCORES):
        b, s = c // 4, c % 4
        qpos = s + 4 * np.arange(QB)                                 # (512,)
        # maskD[i, kt, w]: kt even: query col j=64*(kt//2)+w (w<32);
        # kt odd: j=64*(kt//2)+w (w<64). value = 1 if key<=query pos.
        maskD = np.ones((P, T // P, 64), np.float32)
        for kt in range(T // P):
            m = kt // 2
            wmax = 32 if kt % 2 == 0 else 64
            keys = 128 * kt + np.arange(P)[:, None]                  # (128,1)
            js = 64 * m + np.arange(wmax)[None, :]                   # (1,w)
            maskD[:, kt, :wmax] = (keys <= (s + 4 * js)).astype(np.float32)
        in_maps.append({
            "xT": xTb[b],
            "xqT": np.ascontiguousarray(xTb[b][:, qpos]),
            "wq8m": wq8m.view(np.uint8), "wq8r": wq8r.view(np.uint8),
            "wk8m": wk8m.view(np.uint8), "wk8r": wk8r.view(np.uint8),
            "wv8": wv8.view(np.uint8), "wo8": wo8.view(np.uint8),
            "w18": w18.view(np.uint8), "w28": w28.view(np.uint8),
            "coskT": coskT, "sinkT": sinkT,
            "cosqT": np.ascontiguousarray(coskT[:, qpos]),
            "sinqT": np.ascontiguousarray(sinkT[:, qpos]),
            "maskD": maskD.astype(Bb),
            "ident": ident,
        })
    return in_maps


# ---------------------------------------------------------------- device code
@with_exitstack
def decoder_kernel(ctx, tc):
    nc = tc.nc
    xT_d = nc.dram_tensor("xT", [D, T], F32, kind="ExternalInput").ap()
    xqT_d = nc.dram_tensor("xqT", [D, QB], F32, kind="ExternalInput").ap()
    wq8m_d = nc.dram_tensor("wq8m", [P, NKP, NDT, 2, P], U8, kind="ExternalInput").ap()
    wq8r_d = nc.dram_tensor("wq8r", [P, NKP, NDT, 2, P], U8, kind="ExternalInput").ap()
    wk8m_d = nc.dram_tensor("wk8m", [P, NKP, NDT, 2, P], U8, kind="ExternalInput").ap()
    wk8r_d = nc.dram_tensor("wk8r", [P, NKP, NDT, 2, P], U8, kind="ExternalInput").ap()
    wv8_d = nc.dram_tensor("wv8", [P, NKP, NDT, 2, P], U8, kind="ExternalInput").ap()
    wo8_d = nc.dram_tensor("wo8", [P, NKP, NDT, 2, P], U8, kind="ExternalInput").ap()
    w18_d = nc.dram_tensor("w18", [P, NKP, 2 * FF // P, 2, P], U8, kind="ExternalInput").ap()
    w28_d = nc.dram_tensor("w28", [P, FF // 256, NDT, 2, P], U8, kind="ExternalInput").ap()
    cosk_d = nc.dram_tensor("coskT", [P, T], BF16, kind="ExternalInput").ap()
    sink_d = nc.dram_tensor("sinkT", [P, T], BF16, kind="ExternalInput").ap()
    cosq_d = nc.dram_tensor("cosqT", [P, QB], BF16, kind="ExternalInput").ap()
    sinq_d = nc.dram_tensor("sinqT", [P, QB], BF16, kind="ExternalInput").ap()
    maskD_d = nc.dram_tensor("maskD", [P, T // P, 64], BF16, kind="ExternalInput").ap()
    ident_d = nc.dram_tensor("ident", [P, P], BF16, kind="ExternalInput").ap()
    outT_d = nc.dram_tensor("outT", [D, QB], F32, kind="ExternalOutput").ap()

    MUL = mybir.AluOpType.mult
    ADD = mybir.AluOpType.add
    NTT = T // P            # 16 key tiles
    NM = NTT // 2           # 8 key-tile pairs

    def dr(out, lhsT, rhs, start, stop):
        nc.tensor.matmul(out, lhsT, rhs, start=start, stop=stop,
                         perf_mode=DR, skip_group_check=True)

    # ---------------- persistent pool (survives to the end)
    pers = ctx.enter_context(tc.tile_pool(name="pers", bufs=1))
    maskD = pers.tile([P, NTT, 64], BF16, name="maskD", tag="maskD")
    nc.sync.dma_start(maskD[:], maskD_d[:])
    a8 = [pers.tile([P, 2, QB], FP8, name=f"a8{k}", tag=f"a8{k}")
          for k in range(NKP)]
    hT = [pers.tile([P, QB], F32, name=f"hT{k}", tag=f"hT{k}")
          for k in range(NDT)]
    wo8 = [pers.tile([P, NDT, 2, P], FP8, name=f"wo8{k}", tag=f"wo8{k}")
           for k in range(NKP)]

    # ---------------- attention-lifetime pool
    with tc.tile_pool(name="attw", bufs=1) as attw:
        kT = [attw.tile([P, T], BF16, name=f"kT{k}", tag=f"kT{k}")
              for k in range(NDT)]
        qT = [attw.tile([P, QB], BF16, name=f"qT{k}", tag=f"qT{k}")
              for k in range(NDT)]
        va = [attw.tile([P, H, 2, 80], FP8, name=f"va{m}", tag=f"va{m}")
              for m in range(NM)]
        rstdT = attw.tile([P, NTT], F32, name="rstdT", tag="rstdT")

        # ================= phase 0+1: stream x, norm-stats, projections
        with tc.tile_pool(name="pA", bufs=1) as pA:
            cosk = pA.tile([P, T], BF16, name="cosk", tag="cosk")
            sink = pA.tile([P, T], BF16, name="sink", tag="sink")
            cosq = pA.tile([P, QB], BF16, name="cosq", tag="cosq")
            sinq = pA.tile([P, QB], BF16, name="sinq", tag="sinq")
            ident = pA.tile([P, P], BF16, name="ident", tag="ident")
            nc.sync.dma_start(cosk[:], cosk_d[:])
            nc.sync.dma_start(sink[:], sink_d[:])
            nc.sync.dma_start(cosq[:], cosq_d[:])
            nc.sync.dma_start(sinq[:], sinq_d[:])
            nc.sync.dma_start(ident[:], ident_d[:])
            xv = [pA.tile([P, NTT, 2, P], FP8, name=f"xv{k}", tag=f"xv{k}")
                  for k in range(NKP)]
            xvf = [t[:].rearrange("p a j i -> p j a i") for t in xv]
            xqv = [pA.tile([P, 4, 2, P], FP8, name=f"xqv{k}", tag=f"xqv{k}")
                   for k in range(NKP)]
            xqvf = [t[:].rearrange("p a j i -> p j a i") for t in xqv]
            ones8 = pA.tile([P, 2, 16], FP8, name="ones8", tag="ones8")
            nc.vector.memset(ones8[:], 1.0)

            # ---- stream x pairs: cast fp8, square, ssq via ones-DR-matmul
            with tc.tile_pool(name="ph0s", bufs=2) as ph0s, \
                 tc.tile_pool(name="ph0", bufs=1) as ph0, \
                 tc.tile_pool(name="ph0p", bufs=1, space="PSUM") as ph0p:
                ssq = [ph0p.tile([16, QB], F32, name=f"ssq{tb}", tag=f"ssq{tb}")
                       for tb in range(4)]
                ssqq = ph0p.tile([16, QB], F32, name="ssqq", tag="ssqq")
                for kp in range(NKP):
                    xf = ph0s.tile([P, 2, T], F32, name="xf", tag="xf")
                    src = xT_d[256 * kp:256 * (kp + 1), :] \
                        .rearrange("(j p) t -> p j t", p=P)
                    nc.sync.dma_start(xf[:], src)
                    nc.vector.tensor_copy(xvf[kp], xf[:].rearrange("p j (a i) -> p j a i", i=P))
                    xsq = ph0.tile([P, 2, T], FP8, name="xsq",
                                   tag=f"xsq{kp % 2}")
                    nc.scalar.activation(xsq[:].rearrange("p j (a i) -> p j a i", i=P), xvf[kp], AF.Square)
                    for tb in range(4):
                        dr(ssq[tb][:], ones8[:], xsq[:, :, QB * tb:QB * (tb + 1)],
                           start=(kp == 0), stop=(kp == NKP - 1))
                    # query-block x: cast + square + ssq
                    xqf = ph0s.tile([P, 2, QB], F32, name="xqf", tag="xqf")
                    nc.sync.dma_start(
                        xqf[:], xqT_d[256 * kp:256 * (kp + 1), :]
                        .rearrange("(j p) t -> p j t", p=P))
                    nc.vector.tensor_copy(xqvf[kp], xqf[:].rearrange("p j (a i) -> p j a i", i=P))
                    xqsq = ph0.tile([P, 2, QB], FP8, name="xqsq",
                                    tag=f"xqsq{kp % 2}")
                    nc.scalar.activation(xqsq[:].rearrange("p j (a i) -> p j a i", i=P), xqvf[kp], AF.Square)
                    dr(ssqq[:], ones8[:], xqsq[:],
                       start=(kp == 0), stop=(kp == NKP - 1))

                # rstd rows (f32): rstd = sqrt(1/(ssq/D + eps))
                rstd_row = ph0.tile([1, T], F32, name="rstd_row", tag="rstd_row")
                for tb in range(4):
                    seg = rstd_row[:, QB * tb:QB * (tb + 1)]
                    nc.vector.tensor_scalar(seg, ssq[tb][0:1, :], 1.0 / D, EPS,
                                            op0=MUL, op1=ADD)
                    nc.vector.reciprocal(seg, seg)
                    nc.scalar.activation(seg, seg, AF.Sqrt)
                rstdq_row = ph0.tile([1, QB], F32, name="rstdq_row",
                                     tag="rstdq_row")
                nc.vector.tensor_scalar(rstdq_row[:], ssqq[0:1, :], 1.0 / D, EPS,
                                        op0=MUL, op1=ADD)
                nc.vector.reciprocal(rstdq_row[:], rstdq_row[:])
                nc.scalar.activation(rstdq_row[:], rstdq_row[:], AF.Sqrt)

                # broadcast + fold into rope tables (with 1/SW weight descale)
                rstd_bc = ph0.tile([P, T], F32, name="rstd_bc", tag="rstd_bc")
                nc.gpsimd.partition_broadcast(rstd_bc[:], rstd_row[:])
                rstdq_bc = ph0.tile([P, QB], F32, name="rstdq_bc",
                                    tag="rstdq_bc")
                nc.gpsimd.partition_broadcast(rstdq_bc[:], rstdq_row[:])
                nc.vector.scalar_tensor_tensor(cosk[:], cosk[:], 1.0 / SW,
                                               rstd_bc[:], op0=MUL, op1=MUL)
                nc.vector.scalar_tensor_tensor(sink[:], sink[:], 1.0 / SW,
                                               rstd_bc[:], op0=MUL, op1=MUL)
                nc.vector.scalar_tensor_tensor(cosq[:], cosq[:], 1.0 / SW,
                                               rstdq_bc[:], op0=MUL, op1=MUL)
                nc.vector.scalar_tensor_tensor(sinq[:], sinq[:], 1.0 / SW,
                                               rstdq_bc[:], op0=MUL, op1=MUL)
                # rstdT column layout via PE transpose of cosk (rows 32:64
                # carry rstd/SW exactly: cos=1 there)
                with tc.tile_pool(name="trp", bufs=2, space="PSUM") as trp:
                    for tt in range(NTT):
                        tr = trp.tile([P, P], BF16, name="tr", tag="tr")
                        nc.tensor.transpose(tr[:], cosk[:, P * tt:P * (tt + 1)],
                                            ident[:])
                        nc.vector.tensor_copy(rstdT[:, tt:tt + 1],
                                              tr[:, 32:33])

            # ---- Q projection + rotary (DR fp8, pm folded into wq8r)
            with tc.tile_pool(name="wqp", bufs=1) as wqp, \
                 tc.tile_pool(name="rsc", bufs=3) as rsc, \
                 tc.tile_pool(name="qp", bufs=2, space="PSUM") as qp:
                wqm = [wqp.tile([P, NDT, 2, P], FP8, name=f"wqm{k}", tag=f"wqm{k}")
                       for k in range(NKP)]
                wqr = [wqp.tile([P, NDT, 2, P], FP8, name=f"wqr{k}", tag=f"wqr{k}")
                       for k in range(NKP)]
                for kp in range(NKP):
                    nc.sync.dma_start(wqm[kp][:], wq8m_d[:, kp].bitcast(FP8))
                    nc.sync.dma_start(wqr[kp][:], wq8r_d[:, kp].bitcast(FP8))
                for dt in range(NDT):
                    mps = qp.tile([P, QB], F32, name="mps", tag="mps")
                    rps = qp.tile([P, QB], F32, name="rps", tag="rps")
                    for kp in range(NKP):
                        dr(mps[:], wqm[kp][:, dt, :, :],
                           xqv[kp][:].rearrange("p a j i -> p j a i"),
                           start=(kp == 0), stop=(kp == NKP - 1))
                    for kp in range(NKP):
                        dr(rps[:], wqr[kp][:, dt, :, :],
                           xqv[kp][:].rearrange("p a j i -> p j a i"),
                           start=(kp == 0), stop=(kp == NKP - 1))
                    t1 = rsc.tile([P, QB], F32, name="t1", tag="t1")
                    t2 = rsc.tile([P, QB], F32, name="t2", tag="t2")
                    nc.vector.tensor_mul(t1[:], mps[:], cosq[:])
                    nc.vector.tensor_mul(t2[:], rps[:], sinq[:])
                    nc.vector.tensor_add(qT[dt][:], t1[:], t2[:])

            # ---- K projection + rotary
            with tc.tile_pool(name="wkp", bufs=1) as wkp, \
                 tc.tile_pool(name="rsck", bufs=3) as rsck, \
                 tc.tile_pool(name="kp_", bufs=2, space="PSUM") as kpp:
                wkm = [wkp.tile([P, NDT, 2, P], FP8, name=f"wkm{k}", tag=f"wkm{k}")
                       for k in range(NKP)]
                wkr = [wkp.tile([P, NDT, 2, P], FP8, name=f"wkr{k}", tag=f"wkr{k}")
                       for k in range(NKP)]
                for kp in range(NKP):
                    nc.sync.dma_start(wkm[kp][:], wk8m_d[:, kp].bitcast(FP8))
                    nc.sync.dma_start(wkr[kp][:], wk8r_d[:, kp].bitcast(FP8))
                for dt in range(NDT):
                    for tb in range(4):
                        sl = slice(QB * tb, QB * (tb + 1))
                        mps = kpp.tile([P, QB], F32, name="mpsk", tag="mpsk")
                        rps = kpp.tile([P, QB], F32, name="rpsk", tag="rpsk")
                        xrhs = xv[kp][:, 4 * tb:4 * (tb + 1), :, :] \
                            .rearrange("p a j i -> p j a i")
                        for kp in range(NKP):
                            xrhs = xv[kp][:, 4 * tb:4 * (tb + 1), :, :] \
                                .rearrange("p a j i -> p j a i")
                            dr(mps[:], wkm[kp][:, dt, :, :], xrhs,
                               start=(kp == 0), stop=(kp == NKP - 1))
                        for kp in range(NKP):
                            xrhs = xv[kp][:, 4 * tb:4 * (tb + 1), :, :] \
                                .rearrange("p a j i -> p j a i")
                            dr(rps[:], wkr[kp][:, dt, :, :], xrhs,
                               start=(kp == 0), stop=(kp == NKP - 1))
                        t1 = rsck.tile([P, QB], F32, name="t1k", tag="t1k")
                        t2 = rsck.tile([P, QB], F32, name="t2k", tag="t2k")
                        nc.vector.tensor_mul(t1[:], mps[:], cosk[:, sl])
                        nc.vector.tensor_mul(t2[:], rps[:], sink[:, sl])
                        nc.vector.tensor_add(kT[dt][:, sl], t1[:], t2[:])

            # ---- V projection (natural layout) + rstd scale + ones aug
            with tc.tile_pool(name="wvp", bufs=1) as wvp, \
                 tc.tile_pool(name="vp", bufs=3, space="PSUM") as vpp:
                wv = [wvp.tile([P, NDT, 2, P], FP8, name=f"wv{k}", tag=f"wv{k}")
                      for k in range(NKP)]
                for kp in range(NKP):
                    nc.sync.dma_start(wv[kp][:], wv8_d[:, kp].bitcast(FP8))
                for m in range(NM):
                    nc.vector.memset(va[m][:, :, :, KS:KS + 1], 1.0)
                    nc.vector.memset(va[m][:, :, :, KS + 1:80], 0.0)
                    for j in range(2):
                        tt = 2 * m + j
                        for ch in range(2):
                            vps = vpp.tile([P, QB], F32, name="vps",
                                           tag="vps")
                            wrhs = wv[kp][:, 4 * ch:4 * (ch + 1), :, :]
                            for kp in range(NKP):
                                wrhs = wv[kp][:, 4 * ch:4 * (ch + 1), :, :] \
                                    .rearrange("p a j i -> p j a i")
                                dr(vps[:], xv[kp][:, tt, :, :], wrhs,
                                   start=(kp == 0), stop=(kp == NKP - 1))
                            v3 = vps[:].rearrange("p (h e) -> p h e", e=KS)
                            nc.vector.tensor_scalar(
                                va[m][:, 8 * ch:8 * (ch + 1), j, 0:KS],
                                v3, rstdT[:, tt:tt + 1], None, op0=MUL)
        # pA closed: xr/xq8/tables freed

        # ================= phase 2: attention
        # em[m]: fp8 pair [128, 2, Wm], Wm = 512-64m; query col j = 64m + w
        with tc.tile_pool(name="emp", bufs=2) as emp, \
             tc.tile_pool(name="sps", bufs=5, space="PSUM") as spsp, \
             tc.tile_pool(name="avp", bufs=2, space="PSUM") as avp, \
             tc.tile_pool(name="dnm", bufs=3) as dnm:
            # prefetch O-proj weights while attention runs (pers tiles)
            for k in range(NKP):
                nc.sync.dma_start(wo8[k][:], wo8_d[:, k].bitcast(FP8))

            for h in range(H):
                dt, row = h // 2, (h % 2) * KS
                avps = avp.tile([80, QB], F32, name="avps", tag="avps")
                for m in range(NM):
                    Wm = QB - 64 * m
                    q0 = 64 * m
                    em = emp.tile([P, 2, QB], FP8, name=f"em{m}",
                                  tag=f"em{m % 2}")
                    for j in range(2):
                        kt = 2 * m + j
                        sps = spsp.tile([P, QB], F32, name="sps", tag="sps")
                        nc.tensor.matmul(
                            sps[:, 0:Wm],
                            kT[dt][row:row + KS, P * kt:P * (kt + 1)],
                            qT[dt][row:row + KS, q0:QB],
                            start=True, stop=True, skip_group_check=True)
                        nc.scalar.activation(em[:, j, 0:Wm], sps[:, 0:Wm],
                                             AF.Exp)
                        wmax = 32 if j == 0 else 64
                        nc.gpsimd.tensor_mul(em[:, j, 0:wmax],
                                             em[:, j, 0:wmax],
                                             maskD[:, kt, 0:wmax])
                    dr(avps[:, q0:QB], va[m][:, h, :, :], em[:, :, 0:Wm],
                       start=(m == 0), stop=(m == NM - 1))
                rec = dnm.tile([1, QB], F32, name="rec", tag="rec")
                nc.vector.reciprocal(rec[:], avps[KS:KS + 1, :])
                recb = dnm.tile([KS, QB], F32, name="recb", tag="recb")
                nc.gpsimd.partition_broadcast(recb[:], rec[:])
                # write attention out directly as fp8 pairs for O-proj
                nc.vector.tensor_mul(
                    a8[dt // 2][64 * (h % 2):64 * (h % 2) + KS, dt % 2, :],
                    avps[0:KS, :], recb[:])
    # attw closed: kT/qT/va freed

    # ================= phase 3: O-proj + residual + h-norm
    with tc.tile_pool(name="p3", bufs=1) as p3, \
         tc.tile_pool(name="p3p", bufs=2, space="PSUM") as p3p, \
         tc.tile_pool(name="p3s", bufs=1, space="PSUM") as p3s:
        w2t = [p3.tile([P, NDT, 2, P], FP8, name=f"w2t{k}", tag=f"w2t{k}")
               for k in range(FF // 256)]
        for k in range(FF // 256):
            nc.sync.dma_start(w2t[k][:], w28_d[:, k].bitcast(FP8))
        xq_res = [p3.tile([P, QB], F32, name=f"xqr{k}", tag=f"xqr{k}")
                  for k in range(NDT)]
        for k in range(NDT):
            nc.sync.dma_start(xq_res[k][:], xqT_d[P * k:P * (k + 1), :])
        for dt in range(NDT):
            ops = p3p.tile([P, QB], F32, name="ops", tag="ops")
            for kp in range(NKP):
                dr(ops[:], wo8[kp][:, dt, :, :], a8[kp][:],
                   start=(kp == 0), stop=(kp == NKP - 1))
            nc.vector.scalar_tensor_tensor(hT[dt][:], ops[:], 1.0 / SW,
                                           xq_res[dt][:], op0=MUL, op1=ADD)
        # h-norm -> hn8 fp8 pairs
        hn8 = [p3.tile([P, 2, QB], FP8, name=f"hn8{k}", tag=f"hn8{k}")
               for k in range(NKP)]
        ones8b = p3.tile([P, 2, 16], FP8, name="ones8b", tag="ones8b")
        nc.vector.memset(ones8b[:], 1.0)
        hsq = [p3.tile([P, 2, QB], FP8, name=f"hsq{k % 2}", tag=f"hsq{k % 2}")
               for k in range(2)]
        ssqh = p3s.tile([16, QB], F32, name="ssqh", tag="ssqh")
        for kp in range(NKP):
            hs = hsq[kp % 2]
            for j in range(2):
                nc.scalar.activation(hs[:, j, :], hT[2 * kp + j][:], AF.Square)
            dr(ssqh[:], ones8b[:], hs[:], start=(kp == 0),
               stop=(kp == NKP - 1))
        rsth = p3.tile([1, QB], F32, name="rsth", tag="rsth")
        nc.vector.tensor_scalar(rsth[:], ssqh[0:1, :], 1.0 / D, EPS,
                                op0=MUL, op1=ADD)
        nc.vector.reciprocal(rsth[:], rsth[:])
        nc.scalar.activation(rsth[:], rsth[:], AF.Sqrt)
        rsthb = p3.tile([P, QB], F32, name="rsthb", tag="rsthb")
        nc.gpsimd.partition_broadcast(rsthb[:], rsth[:])
        for kp in range(NKP):
            for j in range(2):
                nc.vector.tensor_mul(hn8[kp][:, j, :], hT[2 * kp + j][:],
                                     rsthb[:])

        # ================= phase 4: FFN (SwiGLU, DR fp8)
        with tc.tile_pool(name="w1p", bufs=2) as w1p, \
             tc.tile_pool(name="gp_", bufs=1) as gpp, \
             tc.tile_pool(name="p4", bufs=2, space="PSUM") as p4, \
             tc.tile_pool(name="p4s", bufs=3) as p4s:
            gp = [gpp.tile([P, 2, QB], FP8, name=f"gp{k}", tag=f"gp{k}")
                  for k in range(FF // 256)]
            for g in range(2):
                w1g = [w1p.tile([P, 32, 2, P], FP8, name=f"w1g{k}",
                                tag=f"w1g{k}") for k in range(NKP)]
                for kp in range(NKP):
                    nc.sync.dma_start(
                        w1g[kp][:],
                        w18_d[:, kp, 32 * g:32 * (g + 1)].bitcast(FP8))
                for fl in range(FF // P // 2):          # 16 f-tiles per group
                    f = g * 16 + fl
                    h1 = p4.tile([P, QB], F32, name="h1", tag="h1")
                    h2 = p4.tile([P, QB], F32, name="h2", tag="h2")
                    for kp in range(NKP):
                        dr(h1[:], w1g[kp][:, 2 * fl, :, :],
                           hn8[kp][:], start=(kp == 0), stop=(kp == NKP - 1))
                    for kp in range(NKP):
                        dr(h2[:], w1g[kp][:, 2 * fl + 1, :, :],
                           hn8[kp][:], start=(kp == 0), stop=(kp == NKP - 1))
                    s1 = p4s.tile([P, QB], BF16, name="s1", tag="s1")
                    nc.scalar.activation(s1[:], h1[:], AF.Silu, scale=1.0 / SW)
                    # gp stores g*GS in fp8
                    nc.vector.scalar_tensor_tensor(
                        gp[f // 2][:, f % 2, :], h2[:], GS / SW, s1[:],
                        op0=MUL, op1=MUL)
        # W2 + residual + store
        with tc.tile_pool(name="p4o", bufs=2, space="PSUM") as p4o, \
             tc.tile_pool(name="p4os", bufs=2) as p4os:
            for dt in range(NDT):
                fps = p4o.tile([P, QB], F32, name="fps", tag="fps")
                for k in range(FF // 256):
                    dr(fps[:], w2t[k][:, dt, :, :], gp[k][:],
                       start=(k == 0), stop=(k == FF // 256 - 1))
                o = p4os.tile([P, QB], F32, name="o", tag="o")
                nc.vector.scalar_tensor_tensor(o[:], fps[:], 1.0 / (SW * GS),
                                               hT[dt][:], op0=MUL, op1=ADD)
                nc.sync.dma_start(outT_d[P * dt:P * (dt + 1), :], o[:])


# ---------------------------------------------------------------- driver
_CACHE = {}


def build_nc():
    if "nc" in _CACHE:
        return _CACHE["nc"]
    nc = bacc.Bacc("TRN2", target_bir_lowering=False, debug=False,
                   enable_asserts=False)
    with tile.TileContext(nc) as tc:
        decoder_kernel(tc)
    nc.compile()
    _CACHE["nc"] = nc
    return nc


def kernel(x, attention_mask, Wq, Wk, Wv, Wo, attn_scale, ffn_scale, W1, W2,
           trace=False):
    from concourse import bass_utils
    in_maps = prep_inputs(x, attention_mask, Wq, Wk, Wv, Wo,
                          attn_scale, ffn_scale, W1, W2)
    nc = build_nc()
    res = bass_utils.run_bass_kernel_spmd(nc, in_maps,
                                          core_ids=list(range(NCORES)),
                                          trace=trace)
    out = np.empty((B, T, D), np.float32)
    for c in range(NCORES):
        b, s = c // 4, c % 4
        out[b, s::4, :] = res.results[c]["outT"].T
    _CACHE["last_result"] = res
    return out
